# Initial kernel scaffold
#
"""Trainium2 Bass kernel for attribute visual attention.

Computes, for each batch b:
    q      = v @ W_alpha                  # [i, f]
    scores = q @ vf[b]                    # [i, r]
    atten  = softmax(scores, axis=r)
    out[b] = atten @ vf[b].T              # [i, f]

Sharding: data-parallel over batch b across 8 NeuronCores (8 batches per
core); v / W_alpha replicated. All matmuls run in fp16 (full PE rate on
TRN2) with fp32 PSUM accumulation; softmax statistics in fp32.

Layout trick: the attend matmul contracts over r, which therefore must
live on SBUF partitions for both operands. The host passes
visual_features twice: once as [f, r] (for the scores matmul) and once
pre-transposed as [r, f] (for the attend matmul). The small attention
matrix itself is transposed on-chip with the DMA xbar (fp16).
"""

import numpy as np
from contextlib import ExitStack

import concourse.bass as bass
import concourse.tile as tile
import concourse.bass_utils as bass_utils
from concourse import bacc, mybir

# Problem shapes (hardcoded per contest contract).
B, F, R, I, V = 64, 2048, 196, 312, 300
NCORES = 8
BL = B // NCORES          # 8 batches per core
FT = F // 128             # 16 f-tiles
RPAD = 256                # r padded to 2x128 for the xbar transpose
I_TILES = ((0, 128), (128, 128), (256, 56))
KV_TILES = ((0, 128), (128, 128), (256, 44))    # v=300
KR_TILES = ((0, 128), (128, 68))                # r=196

F16 = mybir.dt.float16
F32 = mybir.dt.float32

_CACHE = {}


def _build_body(nc, tc, ctx, wa, vt, vf, vft, out):
    const = ctx.enter_context(tc.tile_pool(name="const", bufs=1))
    qtp = ctx.enter_context(tc.tile_pool(name="qt", bufs=1))

    # ---- Phase 0: qT[f, i] = (v @ W_alpha).T via lhsT=W_alpha, rhs=v.T ----
    wa_t, vt_t = [], []
    for k, (v0, vs) in enumerate(KV_TILES):
        w = const.tile([vs, F], F16, tag=f"wa{k}")
        for c in range(4):
            nc.sync.dma_start(w[:, c * 512:(c + 1) * 512],
                              wa[v0:v0 + vs, c * 512:(c + 1) * 512])
        wa_t.append(w)
        t = const.tile([vs, I], F16, tag=f"vt{k}")
        nc.sync.dma_start(t[:], vt[v0:v0 + vs, :])
        vt_t.append(t)

    qt_t = []
    with tc.tile_pool(name="qpsum", bufs=2, space=bass.MemorySpace.PSUM) as qpsum:
        for mf in range(FT):
            qp = qpsum.tile([128, I], F32, tag="qp")
            for k, (v0, vs) in enumerate(KV_TILES):
                nc.tensor.matmul(qp[:], wa_t[k][:, mf * 128:(mf + 1) * 128],
                                 vt_t[k][:], start=(k == 0), stop=(k == 2))
            q = qtp.tile([128, I], F16, tag=f"qt{mf}")
            nc.scalar.copy(q[:], qp[:])
            qt_t.append(q)

    # ---- Phase 1: per-batch attention ----
    vfp = ctx.enter_context(tc.tile_pool(name="vf", bufs=2))
    vftp = ctx.enter_context(tc.tile_pool(name="vft", bufs=2))
    esp = ctx.enter_context(tc.tile_pool(name="es", bufs=3))
    attp = ctx.enter_context(tc.tile_pool(name="atT", bufs=2))
    outp = ctx.enter_context(tc.tile_pool(name="out", bufs=2))
    stat = ctx.enter_context(tc.tile_pool(name="stat", bufs=4))
    spsum = ctx.enter_context(
        tc.tile_pool(name="spsum", bufs=2, space=bass.MemorySpace.PSUM))
    opsum = ctx.enter_context(
        tc.tile_pool(name="opsum", bufs=4, space=bass.MemorySpace.PSUM))

    for b in range(BL):
        vf_t = vfp.tile([128, FT, R], F16, tag="vf")
        for c in range(4):
            nc.sync.dma_start(vf_t[:, c * 4:(c + 1) * 4, :],
                              vf[b, :, c * 4:(c + 1) * 4, :])
        vft_t = []
        for kr, (r0, rs) in enumerate(KR_TILES):
            vv = vftp.tile([rs, F], F16, tag=f"vft{kr}")
            for c in range(2):
                nc.sync.dma_start(vv[:, c * 1024:(c + 1) * 1024],
                                  vft[b, r0:r0 + rs, c * 1024:(c + 1) * 1024])
            vft_t.append(vv)

        for mi, (i0, isz) in enumerate(I_TILES):
            # scores[i, r] for this i-tile: lhsT = qT[f, i], rhs = vf[f, r]
            sp = spsum.tile([isz, R], F32, tag="sp")
            for kf in range(FT):
                nc.tensor.matmul(sp[:], qt_t[kf][:, i0:i0 + isz],
                                 vf_t[:, kf, :],
                                 start=(kf == 0), stop=(kf == FT - 1))

            # softmax over r (free axis): e = exp(s - max), sum via accum_out
            negmax = stat.tile([isz, 1], F32, tag="negmax")
            nc.vector.tensor_reduce(negmax[:], sp[:],
                                    axis=mybir.AxisListType.X,
                                    op=mybir.AluOpType.max, negate=True)
            psz = 128 if isz == 128 else 64   # xbar needs p_dim % 16 == 0
            es = esp.tile([128, RPAD], F16, tag="es")
            nc.vector.memset(es[:psz, R:RPAD], 0.0)
            if isz < psz:
                nc.vector.memset(es[isz:psz, 0:R], 0.0)
            sums = stat.tile([isz, 1], F32, tag="sums")
            nc.scalar.activation(es[:isz, 0:R], sp[:],
                                 mybir.ActivationFunctionType.Exp,
                                 bias=negmax[:], scale=1.0,
                                 accum_out=sums[:])
            rcp = stat.tile([isz, 1], F32, tag="rcp")
            nc.vector.reciprocal(rcp[:], sums[:])

            # transpose e -> eT[r, i] chunks with the DMA xbar
            esT = []
            for kr in range(2):
                tt = attp.tile([128, psz], F16, tag=f"esT{kr}{mi}")
                nc.sync.dma_start(tt[:], es[0:psz, kr * 128:(kr + 1) * 128],
                                  transpose=True)
                esT.append(tt)

            # attend: out[i, f] = (eT.T @ vfT) * rcp  (normalization fused
            # into the PSUM->SBUF copy as a per-partition scale)
            ot = outp.tile([isz, F], F16, tag=f"ot{mi}")
            for nf in range(4):
                op_ = opsum.tile([isz, 512], F32, tag="op")
                nc.tensor.matmul(op_[:], esT[0][:, 0:isz],
                                 vft_t[0][:, nf * 512:(nf + 1) * 512],
                                 start=True, stop=False)
                nc.tensor.matmul(op_[:], esT[1][0:68, 0:isz],
                                 vft_t[1][:, nf * 512:(nf + 1) * 512],
                                 start=False, stop=True)
                if nf % 2 == 0:
                    nc.scalar.mul(ot[:, nf * 512:(nf + 1) * 512], op_[:],
                                  mul=rcp[:])
                else:
                    nc.vector.tensor_scalar_mul(
                        ot[:, nf * 512:(nf + 1) * 512], op_[:], rcp[:])
            for c in range(2):
                nc.sync.dma_start(out[b, i0:i0 + isz, c * 1024:(c + 1) * 1024],
                                  ot[:, c * 1024:(c + 1) * 1024])


def _get_program():
    if "nc" in _CACHE:
        return _CACHE["nc"]
    nc = bacc.Bacc("TRN2", target_bir_lowering=False, debug=False,
                   num_devices=NCORES)
    wa_d = nc.dram_tensor("walpha", [V, F], F16, kind="ExternalInput")
    vt_d = nc.dram_tensor("vt", [V, I], F16, kind="ExternalInput")
    vf_d = nc.dram_tensor("vf", [BL, 128, FT, R], F16, kind="ExternalInput")
    vft_d = nc.dram_tensor("vft", [BL, R, F], F16, kind="ExternalInput")
    out_d = nc.dram_tensor("out", [BL, I, F], F16, kind="ExternalOutput")

    with tile.TileContext(nc) as tc, ExitStack() as ctx:
        _build_body(nc, tc, ctx, wa_d.ap(), vt_d.ap(), vf_d.ap(),
                    vft_d.ap(), out_d.ap())
    nc.compile()
    _CACHE["nc"] = nc
    return nc


def _prep_inputs(visual_features, v, W_alpha):
    vf = np.asarray(visual_features, dtype=np.float32)
    v = np.asarray(v, dtype=np.float32)
    W = np.asarray(W_alpha, dtype=np.float32)

    walpha16 = np.ascontiguousarray(W).astype(np.float16)          # [V, F]
    vt16 = np.ascontiguousarray(v.T).astype(np.float16)            # [V, I]
    # [b, f, r] -> [b, p=128, t=16, r]: per-partition-contiguous DMA layout
    vf16 = np.ascontiguousarray(
        vf.reshape(B, FT, 128, R).transpose(0, 2, 1, 3)).astype(np.float16)
    vft16 = np.ascontiguousarray(vf.transpose(0, 2, 1)).astype(np.float16)

    in_maps = []
    for c in range(NCORES):
        sl = slice(c * BL, (c + 1) * BL)
        in_maps.append({
            "walpha": walpha16,
            "vt": vt16,
            "vf": np.ascontiguousarray(vf16[sl]),
            "vft": np.ascontiguousarray(vft16[sl]),
        })
    return in_maps


def kernel(visual_features, v, W_alpha, _profile=False):
    nc = _get_program()
    in_maps = _prep_inputs(visual_features, v, W_alpha)
    res = bass_utils.run_bass_kernel_spmd(
        nc, in_maps, core_ids=list(range(NCORES)), trace=_profile)
    outs = [res.results[c]["out"] for c in range(NCORES)]
    full = np.concatenate(outs, axis=0).astype(np.float32)
    if _profile:
        return full, res
    return full


# revision 14
# speedup vs baseline: 1.2940x; 1.2940x over previous
"""Trainium2 Bass kernel for attribute visual attention.

Computes, for each batch b:
    q      = v @ W_alpha                  # [i, f]
    scores = q @ vf[b]                    # [i, r]
    atten  = softmax(scores, axis=r)
    out[b] = atten @ vf[b].T              # [i, f]

Sharding: data-parallel over batch b across 8 NeuronCores (8 batches per
core); v / W_alpha replicated. All matmuls run in fp16 (full PE rate on
TRN2) with fp32 PSUM accumulation; softmax statistics in fp32.

Layout notes:
- The attend matmul contracts over r, which must live on SBUF partitions
  for both operands; the host passes visual_features twice — [f, r] for
  the scores matmul and pre-transposed [r, f] for the attend matmul. The
  small e = exp(scores - max) matrix is transposed on-chip with the DMA
  xbar (fp16).
- Batches are processed in PAIRS for the scores matmul (rhs = two
  batches side by side, N=392): halves the number of PE instructions and
  stationary-weight loads.
- Bulk HBM traffic uses SWDGE (gpsimd) so the shared HWDGE block is left
  for the xbar transposes.
- Softmax normalization is folded into the PSUM->SBUF output copy as a
  per-partition scale.
"""

import numpy as np
from contextlib import ExitStack

import concourse.bass as bass
import concourse.tile as tile
import concourse.bass_utils as bass_utils
from concourse import bacc, mybir

# Problem shapes (hardcoded per contest contract).
B, F, R, I, V = 64, 2048, 196, 312, 300
NCORES = 8
BL = B // NCORES          # 8 batches per core
NPAIR = BL // 2           # 4 batch-pairs per core
FT = F // 128             # 16 f-tiles
RPAD = 256                # r padded to 2x128 for the xbar transpose
I_TILES = ((0, 128), (128, 128), (256, 56))
KV_TILES = ((0, 128), (128, 128), (256, 44))    # v=300
KR_TILES = ((0, 128), (128, 68))                # r=196

F16 = mybir.dt.float16
F32 = mybir.dt.float32

_CACHE = {}


def _build_body(nc, tc, ctx, wa, vt, vf, vft, out, reps):
    qtp = ctx.enter_context(tc.tile_pool(name="qt", bufs=1))

    # ---- Phase 0: qT[f, i] = (v @ W_alpha).T via lhsT=W_alpha, rhs=v.T ----
    qt_t = []
    with tc.tile_pool(name="const", bufs=1) as const, \
         tc.tile_pool(name="qpsum", bufs=2, space=bass.MemorySpace.PSUM) as qpsum:
        wa_t, vt_t = [], []
        for k, (v0, vs) in enumerate(KV_TILES):
            w = const.tile([vs, F], F16, tag=f"wa{k}")
            nc.sync.dma_start(w[:], wa[v0:v0 + vs, :])
            wa_t.append(w)
            t = const.tile([vs, I], F16, tag=f"vt{k}")
            nc.sync.dma_start(t[:], vt[v0:v0 + vs, :])
            vt_t.append(t)

        for mf in range(FT):
            qp = qpsum.tile([128, I], F32, tag="qp")
            for k, (v0, vs) in enumerate(KV_TILES):
                nc.tensor.matmul(qp[:], wa_t[k][:, mf * 128:(mf + 1) * 128],
                                 vt_t[k][:], start=(k == 0), stop=(k == 2))
            q = qtp.tile([128, I], F16, tag=f"qt{mf}")
            nc.scalar.copy(q[:], qp[:])
            qt_t.append(q)

    # ---- Phase 1: per batch-pair attention ----
    vfp = ctx.enter_context(tc.tile_pool(name="vf", bufs=4))
    vftp = ctx.enter_context(tc.tile_pool(name="vft", bufs=3))
    esp = ctx.enter_context(tc.tile_pool(name="es", bufs=4))
    attp = ctx.enter_context(tc.tile_pool(name="atT", bufs=2))
    outp = ctx.enter_context(tc.tile_pool(name="out", bufs=2))
    stat = ctx.enter_context(tc.tile_pool(name="stat", bufs=6))
    spsum = ctx.enter_context(
        tc.tile_pool(name="spsum", bufs=2, space=bass.MemorySpace.PSUM))
    opsum = ctx.enter_context(
        tc.tile_pool(name="opsum", bufs=6, space=bass.MemorySpace.PSUM))

    PW = 1     # pairs per wave
    for rep in range(reps):
        for half in range(NPAIR // PW):
            # vf pair tiles: [128, t, j*196+r] for the wave's batch pairs
            vf_t, vft_t = [], {}
            for p in range(PW):
                bp = half * PW + p
                vt_ = vfp.tile([128, FT, 2 * R], F16, tag="vf", name=f"vf{p}")
                nc.gpsimd.dma_start(vt_[:], vf[bp])
                vf_t.append(vt_)
                for j in range(2):
                    b = 2 * bp + j
                    jj = 2 * p + j
                    for kr, (r0, rs) in enumerate(KR_TILES):
                        vv = vftp.tile([rs, F], F16, tag=f"vft{kr}{jj}",
                                       name=f"vft{kr}{jj}")
                        nc.gpsimd.dma_start(vv[:], vft[b, r0:r0 + rs, :])
                        vft_t[(jj, kr)] = vv

            for mi, (i0, isz) in enumerate(I_TILES):
                # scores for all wave batches; inner loop over pairs so the
                # stationary qT tile is reused PW times per load
                sps = [spsum.tile([isz, 2, R], F32, tag="sp", name=f"sp{p}")
                       for p in range(PW)]
                for kf in range(FT):
                    for p in range(PW):
                        nc.tensor.matmul(
                            sps[p][:], qt_t[kf][:, i0:i0 + isz],
                            vf_t[p][:, kf, :].rearrange("p (j r) -> p j r", j=2),
                            start=(kf == 0), stop=(kf == FT - 1))

                psz = 128 if isz == 128 else 64   # xbar: p_dim % 16 == 0
                for p in range(PW):
                    sp = sps[p]
                    negmax = stat.tile([isz, 2], F32, tag="negmax")
                    nc.vector.tensor_reduce(negmax[:], sp[:],
                                            axis=mybir.AxisListType.X,
                                            op=mybir.AluOpType.max, negate=True)
                    sums = stat.tile([isz, 2], F32, tag="sums")
                    rcp = stat.tile([isz, 2], F32, tag="rcp")
                    for j in range(2):
                        jj = 2 * p + j
                        b = 2 * (half * PW + p) + j
                        es = esp.tile([128, RPAD], F16, tag="es")
                        nc.vector.memset(es[:psz, R:RPAD], 0.0)
                        if isz < psz:
                            # zero pad rows [isz:psz]; SBUF partition starts
                            # are 32-aligned, so zero [32:psz]; the activation
                            # then overwrites [32:isz].
                            nc.vector.memset(es[32:psz, 0:R], 0.0)
                        nc.scalar.activation(es[:isz, 0:R], sp[:, j, :],
                                             mybir.ActivationFunctionType.Exp,
                                             bias=negmax[:, j:j + 1], scale=1.0,
                                             accum_out=sums[:, j:j + 1])
                        nc.vector.reciprocal(rcp[:, j:j + 1], sums[:, j:j + 1])

                        # transpose e -> eT[r, i] chunks with the DMA xbar
                        esT = []
                        for kr in range(2):
                            tt = attp.tile([128, psz], F16, tag=f"esT{kr}{mi}",
                                           name=f"esT{kr}")
                            nc.sync.dma_start(
                                tt[:], es[0:psz, kr * 128:(kr + 1) * 128],
                                transpose=True)
                            esT.append(tt)

                        # attend: out[i, f] = (eT.T @ vfT) * rcp
                        # (normalization fused into the PSUM->SBUF copy as a
                        # per-partition scale). kr outer so 4 consecutive
                        # matmuls share the stationary operand.
                        ot = outp.tile([isz, F], F16, tag=f"ot{mi}{j}", name=f"ot{mi}{j}")
                        ops = [opsum.tile([isz, 512], F32, tag="op",
                                          name=f"op{nf}") for nf in range(4)]
                        for kr in range(2):
                            lhs = (esT[0][:, 0:isz] if kr == 0
                                   else esT[1][0:68, 0:isz])
                            for nf in range(4):
                                nc.tensor.matmul(
                                    ops[nf][:], lhs,
                                    vft_t[(jj, kr)][:, nf * 512:(nf + 1) * 512],
                                    start=(kr == 0), stop=(kr == 1))
                        for nf in range(4):
                            if nf % 2 == 0:
                                nc.scalar.mul(ot[:, nf * 512:(nf + 1) * 512],
                                              ops[nf][:], mul=rcp[:, j:j + 1])
                            else:
                                nc.vector.tensor_scalar_mul(
                                    ot[:, nf * 512:(nf + 1) * 512], ops[nf][:],
                                    rcp[:, j:j + 1])
                        nc.sync.dma_start(out[b, i0:i0 + isz, :], ot[:])


def _get_program(reps=1):
    key = ("nc", reps)
    if key in _CACHE:
        return _CACHE[key]
    nc = bacc.Bacc("TRN2", target_bir_lowering=False, debug=False,
                   num_devices=NCORES)
    wa_d = nc.dram_tensor("walpha", [V, F], F16, kind="ExternalInput")
    vt_d = nc.dram_tensor("vt", [V, I], F16, kind="ExternalInput")
    vf_d = nc.dram_tensor("vf", [NPAIR, 128, FT, 2 * R], F16,
                          kind="ExternalInput")
    vft_d = nc.dram_tensor("vft", [BL, R, F], F16, kind="ExternalInput")
    out_d = nc.dram_tensor("out", [BL, I, F], F16, kind="ExternalOutput")

    with tile.TileContext(nc) as tc, ExitStack() as ctx:
        _build_body(nc, tc, ctx, wa_d.ap(), vt_d.ap(), vf_d.ap(),
                    vft_d.ap(), out_d.ap(), reps)
    nc.compile()
    _CACHE[key] = nc
    return nc


def _prep_inputs(visual_features, v, W_alpha):
    vf = np.asarray(visual_features, dtype=np.float32)
    v = np.asarray(v, dtype=np.float32)
    W = np.asarray(W_alpha, dtype=np.float32)

    walpha16 = np.ascontiguousarray(W).astype(np.float16)          # [V, F]
    vt16 = np.ascontiguousarray(v.T).astype(np.float16)            # [V, I]
    # [b, f, r] -> [bp, p=128, t=16, j*196+r]: batch-paired, per-partition
    # contiguous DMA layout
    vf16 = np.ascontiguousarray(
        vf.reshape(B // 2, 2, FT, 128, R).transpose(0, 3, 2, 1, 4)
        .reshape(B // 2, 128, FT, 2 * R)).astype(np.float16)
    vft16 = np.ascontiguousarray(vf.transpose(0, 2, 1)).astype(np.float16)

    in_maps = []
    for c in range(NCORES):
        in_maps.append({
            "walpha": walpha16,
            "vt": vt16,
            "vf": np.ascontiguousarray(vf16[c * NPAIR:(c + 1) * NPAIR]),
            "vft": np.ascontiguousarray(vft16[c * BL:(c + 1) * BL]),
        })
    return in_maps


def kernel(visual_features, v, W_alpha):
    nc = _get_program()
    in_maps = _prep_inputs(visual_features, v, W_alpha)
    res = bass_utils.run_bass_kernel_spmd(
        nc, in_maps, core_ids=list(range(NCORES)))
    outs = [res.results[c]["out"] for c in range(NCORES)]
    return np.concatenate(outs, axis=0).astype(np.float32)
